# revision 3
# baseline (speedup 1.0000x reference)
"""Contrastive L2 loss (match/non-match descriptor loss) on Trainium2.

Strategy: data-parallel over batch B=8 across 8 NeuronCores (1 image pair per
core).  Per core the kernel:
  * loads the index tensors into single-partition SBUF rows,
  * indirect-DMA gathers the 64B descriptor rows for match (2x5000) and
    non-match (2x50000) indices straight from the HBM-resident outA/outB
    shards (this random 64B-row gather is the memory roofline of the op),
  * computes sum((mA-mB)^2), dist=sqrt(sum((nA-nB)^2, D)), sum(dist),
    nm=relu(mean-dist)^2 sums and the hard-negative count on DVE/ACT,
  * writes 4 partial scalars to DRAM.
Host combines the 8x4 partials into the 3 reference scalars.

Indirect-DMA contract (validated on HW this session): each
indirect_dma_start generates ONE descriptor per dest PARTITION, reading the
dest partition's contiguous byte extent from table[idx[p, 0]].  So the only
working shape is idx [P, 1] column + dest [P, 16] row: 125 descriptors
(64B each) per instruction.  Larger multi-descriptor forms ([1, n] index
streams, [P, n] + 3-D dests) silently scramble or crash on HW even though
CoreSim accepts them.  InstDMAGatherAnt batches descriptor generation but
is capped at 1024 descriptors/instruction and int16 (<=32768-row) sources,
which our 307200-row table only satisfies via a (slab_a, slab_b) pair
bucketing scheme -- measured-feasible (see session notes) but not landed.

Cost structure (TimelineSim, matches HW spec constants): 880 gather
instructions x (994ns SWDGE fixed + 125*0.34ns/desc) = 912us of Pool-engine
descriptor generation; DMA engines only ~50us; DVE ~26us.  This kernel is
at the floor of the per-row SWDGE gather instruction set.

125 partitions are used so 5000 and 50000 both divide evenly -> no padding.
Cross-partition folds go through a tiny SBUF->SBUF reshaping DMA + DVE
reduce (exact fp32; PE is avoided entirely).
"""

import numpy as np

B, N, D = 8, 307200, 16
M, K = 5000, 50000
NON_MATCH_LOSS_WEIGHT = 1.0

P = 125            # partitions used; divides both M and K evenly
MN = M // P        # 40 match rows per partition
KN = K // P        # 400 non-match rows per partition
K_CHUNKS = 5
KC = KN // K_CHUNKS  # rows per partition per chunk (100)
KCHUNK = K // K_CHUNKS  # indices per chunk (12500 descriptors/instruction)

_CACHE = {}


def _build_nc(debug=False):
    import concourse.bacc as bacc
    import concourse.mybir as mybir
    from concourse.bass import AP, IndirectOffsetOnAxis
    from concourse.tile import TileContext


    f32 = mybir.dt.float32
    i32 = mybir.dt.int32
    X = mybir.AxisListType.X
    Alu = mybir.AluOpType
    Act = mybir.ActivationFunctionType

    nc = bacc.Bacc()
    outA = nc.declare_dram_parameter("outA", [N, D], f32, isOutput=False)
    outB = nc.declare_dram_parameter("outB", [N, D], f32, isOutput=False)
    matchA = nc.declare_dram_parameter("matchA", [M], i32, isOutput=False)
    matchB = nc.declare_dram_parameter("matchB", [M], i32, isOutput=False)
    nonMatchA = nc.declare_dram_parameter("nonMatchA", [K], i32, isOutput=False)
    nonMatchB = nc.declare_dram_parameter("nonMatchB", [K], i32, isOutput=False)
    stats = nc.declare_dram_parameter("stats", [1, 4], f32, isOutput=True)

    with TileContext(nc) as tc:
        with (
            tc.tile_pool(name="idx", bufs=1) as idxp,
            tc.tile_pool(name="gather", bufs=2) as gp,
            tc.tile_pool(name="work", bufs=2) as wp,
            tc.tile_pool(name="persist", bufs=1) as pp,
        ):
            # ---- index tensors into SBUF, [P, rowlen] ---------------
            midxA = idxp.tile([P, MN], i32)
            midxB = idxp.tile([P, MN], i32)
            nidxA = idxp.tile([P, KN], i32)
            nidxB = idxp.tile([P, KN], i32)
            nc.sync.dma_start(out=midxA[:], in_=matchA[:].rearrange("(p n) -> p n", p=P))
            nc.sync.dma_start(out=midxB[:], in_=matchB[:].rearrange("(p n) -> p n", p=P))
            nc.sync.dma_start(out=nidxA[:], in_=nonMatchA[:].rearrange("(p n) -> p n", p=P))
            nc.sync.dma_start(out=nidxB[:], in_=nonMatchB[:].rearrange("(p n) -> p n", p=P))

            def gather_cols(dst_tile, table, idx_tile, j0, ncols):
                # production-proven shape: idx [P,1] column, dest [P,16] row
                # per instruction (one descriptor per partition).
                for j in range(j0, j0 + ncols):
                    nc.gpsimd.indirect_dma_start(
                        out=dst_tile[:, (j - j0) * D:(j - j0 + 1) * D],
                        out_offset=None, in_=table,
                        in_offset=IndirectOffsetOnAxis(
                            ap=idx_tile[:, j:j + 1], axis=0))

            # ---- persistent accumulators ------------------------------
            dist = pp.tile([P, KN], f32)          # all non-match distances
            parts = pp.tile([P, 4], f32)          # [match_sq, nm_sum, hn, dist_sum]

            # ---- match part ------------------------------------------
            mA = gp.tile([P, MN * D], f32)
            mB = gp.tile([P, MN * D], f32)
            gather_cols(mA, outA[:], midxA, 0, MN)
            gather_cols(mB, outB[:], midxB, 0, MN)
            mD = wp.tile([P, MN * D], f32, bufs=1)
            nc.vector.tensor_sub(mD[:], mA[:], mB[:])
            mSq = wp.tile([P, MN * D], f32, bufs=1)
            nc.vector.tensor_mul(mSq[:], mD[:], mD[:])
            nc.vector.reduce_sum(out=parts[:, 0:1], in_=mSq[:], axis=X)

            # ---- non-match distances, chunked ------------------------
            for c in range(K_CHUNKS):
                sl = slice(c * KC, (c + 1) * KC)
                nA = gp.tile([P, KC * D], f32, tag="nA")
                nB = gp.tile([P, KC * D], f32, tag="nB")
                gather_cols(nA, outA[:], nidxA, c * KC, KC)
                gather_cols(nB, outB[:], nidxB, c * KC, KC)
                df = wp.tile([P, KC * D], f32, tag="df")
                nc.vector.tensor_sub(df[:], nA[:], nB[:])
                sq = wp.tile([P, KC * D], f32, tag="sq")
                nc.vector.tensor_mul(sq[:], df[:], df[:])
                d2 = wp.tile([P, KC], f32, tag="d2")
                nc.vector.reduce_sum(
                    out=d2[:], in_=sq[:].rearrange("p (n d) -> p n d", d=D), axis=X)
                nc.scalar.activation(out=dist[:, sl], in_=d2[:], func=Act.Sqrt)

            # ---- mean over all K distances ---------------------------
            # per-partition row sums, then fold partitions via a tiny
            # SBUF->SBUF reshaping DMA + DVE reduce (exact fp32).
            nc.vector.reduce_sum(out=parts[:, 3:4], in_=dist[:], axis=X)
            dcol = pp.tile([1, P], f32)
            nc.sync.dma_start(out=dcol[:], in_=parts[:, 3:4])
            meanp0 = pp.tile([1, 1], f32)
            nc.vector.reduce_sum(out=meanp0[:], in_=dcol[:], axis=X)
            nc.scalar.mul(meanp0[:], meanp0[:], 1.0 / K)
            mean_bc = pp.tile([P, 1], f32)
            nc.gpsimd.partition_broadcast(out_ap=mean_bc[:], in_ap=meanp0[:])

            # ---- nm = relu(mean - dist)^2, hn = count(dist < mean) ---
            t = wp.tile([P, KN], f32, bufs=1)
            nc.vector.tensor_scalar(
                out=t[:], in0=dist[:], scalar1=mean_bc[:, 0:1], scalar2=None,
                op0=Alu.subtract)          # t = dist - mean
            nm = wp.tile([P, KN], f32, bufs=1)
            # (t min 0) * t == relu(mean-dist)^2 elementwise
            nc.vector.scalar_tensor_tensor(
                out=nm[:], in0=t[:], scalar=0.0, in1=t[:],
                op0=Alu.min, op1=Alu.mult)
            nc.vector.reduce_sum(out=parts[:, 1:2], in_=nm[:], axis=X)
            ind = wp.tile([P, KN], f32, bufs=1)
            nc.vector.tensor_scalar(
                out=ind[:], in0=t[:], scalar1=0.0, scalar2=None,
                op0=Alu.is_lt)
            nc.vector.reduce_sum(out=parts[:, 2:3], in_=ind[:], axis=X)

            # ---- fold partitions and write out -----------------------
            prow = pp.tile([1, P * 4], f32)
            nc.sync.dma_start(out=prow[:], in_=parts[:])
            stats_row = pp.tile([1, 4], f32)
            # prow layout is partition-major: element [p*4 + c]; view as
            # [1, 4(c, step 1), 125(p, step 4)] and reduce the p axis.
            nc.vector.reduce_sum(
                out=stats_row[:],
                in_=prow[:].rearrange("o (p c) -> o c p", c=4),
                axis=X)
            nc.sync.dma_start(out=stats[:], in_=stats_row[:])

    nc.finalize()
    return nc


def _get_nc():
    if "nc" not in _CACHE:
        _CACHE["nc"] = _build_nc()
    return _CACHE["nc"]


def kernel(outA, outB, matchA, matchB, nonMatchA, nonMatchB, hardNegative):
    from concourse.bass_utils import run_bass_kernel_spmd

    outA = np.asarray(outA, dtype=np.float32)
    outB = np.asarray(outB, dtype=np.float32)
    matchA = np.asarray(matchA, dtype=np.int32)
    matchB = np.asarray(matchB, dtype=np.int32)
    nonMatchA = np.asarray(nonMatchA, dtype=np.int32)
    nonMatchB = np.asarray(nonMatchB, dtype=np.int32)
    hard = int(np.asarray(hardNegative))

    nc = _get_nc()
    in_maps = [
        {
            "outA": np.ascontiguousarray(outA[b]),
            "outB": np.ascontiguousarray(outB[b]),
            "matchA": np.ascontiguousarray(matchA[b]),
            "matchB": np.ascontiguousarray(matchB[b]),
            "nonMatchA": np.ascontiguousarray(nonMatchA[b]),
            "nonMatchB": np.ascontiguousarray(nonMatchB[b]),
        }
        for b in range(B)
    ]
    res = run_bass_kernel_spmd(nc, in_maps, core_ids=list(range(B)))
    stats = np.stack([np.asarray(r["stats"]).reshape(4) for r in res.results])

    match_sq = stats[:, 0].astype(np.float64)
    nm_sum = stats[:, 1].astype(np.float64)
    hn = stats[:, 2].astype(np.float64)
    match_loss = (match_sq / M).astype(np.float32)
    if hard:
        denom = np.where(hn == 0, float(K), hn)
    else:
        denom = np.full(B, float(K))
    nm_loss = (NON_MATCH_LOSS_WEIGHT * nm_sum / denom).astype(np.float32)

    match_sum = np.float32(np.sum(match_loss, dtype=np.float32))
    non_match_sum = np.float32(np.sum(nm_loss, dtype=np.float32))
    return (
        np.float32(match_sum + non_match_sum),
        match_sum,
        non_match_sum,
    )
